# revision 31
# baseline (speedup 1.0000x reference)
"""3-layer GCN (message passing) on 8 NeuronCores via Bass/Tile.

Strategy (vertex-cut / dst-sharding, bf16 data path):
  - Nodes are LPT-packed into (core, block) bins by in-degree so every
    128-dst block needs the same ceil(1404/128)=11 edge chunks on every
    core (SPMD program, minimal gather launches). Output rows are
    un-permuted on the host at the end.
  - Layer 1 is re-associated: relu(A_hat (X W1) + b1) = relu((A_hat X') W1
    + b1) with X' = dinv*X pre-scaled on host and expanded to edge order
    (xe), so L1 needs no indirect gathers at all - pure streaming.
  - Layers 2/3: each core computes its Y = dinv*(h @ W) shard; the table
    is AllGathered in FOUR row-pieces interleaved with the producing loop
    (gather indices are host-remapped to the piece-concatenated physical
    layout), then per-chunk indirect gathers fetch 512B bf16 rows.
    GpSimd SWDGE launch issue (~1.45us per 128-row gather) is the
    critical resource; everything else hides under it. The self-loop
    chunk of each block reads the core's OWN ybin shard rows with one
    cheap direct HWDGE DMA (core-independent address), saving one
    indirect launch per block per layer.
  - Scatter-add realized on TensorE with host-built bf16 one-hot scatter
    matrices S[e, dst] = dinv[dst] streamed per block (sall).
  - L1/L2 scatter runs transposed (psT = G^T @ S) so the ReLU epilogue
    writes h^T directly into the resident xts tile (no transposes);
    bias+relu fused into one ScalarE activation per feature half. The
    next layer's phase-1 window is interleaved after each block.
  - L3 scatter runs direct (ps = S^T @ G) to emit [dst, feat] fp32 rows.
"""

import os
import sys

sys.path.insert(0, "/opt/trn_rl_repo")

import numpy as np
import ml_dtypes

BF16 = ml_dtypes.bfloat16

N = 50000
E = 500000
NC = 8
SH = N // NC            # 6250 nodes per core
P = 128
DIN = 128
DH = 256
NBLK = (SH + P - 1) // P      # 49 dst blocks per core
LASTM = SH - (NBLK - 1) * P   # 106 dsts in the last block
AG_WB = [0, 13, 25, 37, NBLK]          # phase1 window boundaries per AG piece
AG_R = [(AG_WB[q] * P, min(AG_WB[q + 1] * P, SH)) for q in range(4)]


def _balance(deg):
    """LPT-pack nodes into (core, block) bins by in-edge weight so every
    block's edge count is ~equal across cores. Returns perm: node -> device
    row (core*SH + block*128 + slot)."""
    import heapq
    w = deg.astype(np.int64)           # regular in-edges per node
    nodes = np.argsort(-w, kind="stable")
    caps = []
    for c in range(NC):
        for b in range(NBLK):
            cap = LASTM if b == NBLK - 1 else P
            caps.append((c * SH + b * P, cap))
    heap = [(0, i) for i in range(len(caps))]
    heapq.heapify(heap)
    fill = [0] * len(caps)
    perm = np.empty(N, dtype=np.int64)
    for n in nodes:
        while True:
            wt, i = heapq.heappop(heap)
            if fill[i] < caps[i][1]:
                break
        perm[n] = caps[i][0] + fill[i]
        fill[i] += 1
        if fill[i] < caps[i][1]:
            heapq.heappush(heap, (wt + int(w[n]), i))
    return perm


def _preprocess(x, edge_index):
    """Host-side graph partitioning. Returns per-core tensors + layout meta."""
    src = np.asarray(edge_index[0], dtype=np.int64)
    dst = np.asarray(edge_index[1], dtype=np.int64)
    deg = np.bincount(dst, minlength=N).astype(np.float64)
    dinv0 = (1.0 / np.sqrt(deg + 1.0)).astype(np.float32)

    perm = _balance(deg)
    inv = np.empty(N, dtype=np.int64)
    inv[perm] = np.arange(N)           # device row -> original node

    # remap everything into device order
    src = perm[src]
    dst = perm[dst]
    x = np.asarray(x, np.float32)[inv]
    dinv = dinv0[inv]

    order = np.argsort(dst, kind="stable")
    s_s = src[order]
    d_s = dst[order]
    bounds = np.searchsorted(d_s, np.arange(NC + 1) * SH)

    # chunk counts per (core, block); cpb shared across cores (SPMD program)
    cnts = np.zeros((NC, NBLK), dtype=np.int64)
    for c in range(NC):
        lo, hi = bounds[c], bounds[c + 1]
        blk = (d_s[lo:hi] - c * SH) >> 7
        cnts[c] = np.bincount(blk, minlength=NBLK)
    cpb = np.maximum(1, (cnts.max(axis=0) + P - 1) // P) + 1  # + self chunk
    cum = np.concatenate([[0], np.cumsum(cpb)])
    nch = int(cum[-1])

    xs = (dinv[:, None] * np.asarray(x, np.float32)).astype(BF16)  # [N, 128]

    per_core = []
    for c in range(NC):
        lo, hi = bounds[c], bounds[c + 1]
        sc = s_s[lo:hi]
        dc = d_s[lo:hi] - c * SH
        blk = dc >> 7
        n_e = hi - lo
        starts = np.concatenate([[0], np.cumsum(cnts[c])])
        pos = np.arange(n_e) - starts[blk]
        col = cum[blk] + (pos >> 7)     # chunk column
        prow = pos & 127                # partition (edge slot)

        idx_all = np.zeros((P, nch), dtype=np.int32)
        # physical table row under 4-piece split-AllGather layout
        gc = sc // SH
        gr = sc % SH
        phys = np.zeros_like(sc)
        for q in range(4):
            lo, hi = AG_R[q]
            sel = (gr >= lo) & (gr < hi)
            phys[sel] = NC * lo + gc[sel] * (hi - lo) + (gr[sel] - lo)
        idx_all[prow, col] = phys.astype(np.int32)

        # host-built scatter matrices: s_all[p, col*128 + dblk] = dinv[dst]
        s_all = np.zeros((P, nch, P), dtype=np.float32)
        s_all[prow, col, dc & 127] = dinv[dc + c * SH]

        # L1 expanded edge table: xe[p, col*128 + j] = xs[src, j] (0 pads)
        xe = np.zeros((P, nch, DIN), dtype=BF16)
        xe[prow, col, :] = xs[sc]

        # self chunk (last chunk of each block): diagonal S; own rows in xe
        for b in range(NBLK):
            mm = LASTM if b == NBLK - 1 else P
            selfcol = cum[b + 1] - 1
            own = c * SH + b * P + np.arange(mm)
            s_all[np.arange(mm), selfcol, np.arange(mm)] = dinv[own]
            xe[:mm, selfcol, :] = xs[own]
        s_all = s_all.reshape(P, nch * P).astype(BF16)
        xe = xe.reshape(P, nch * DIN)

        # dinv of own shard in [p, w] window layout
        ids = c * SH + np.arange(NBLK * P)
        valid = ids < (c + 1) * SH
        dc_own = np.where(valid, dinv[np.minimum(ids, N - 1)], 0.0)
        dc_own = dc_own.reshape(NBLK, P).T.astype(np.float32).copy()

        per_core.append({
            "idx": idx_all,
            "sall": s_all,
            "xe": xe,
            "dco": dc_own,
        })

    meta = {"cpb": cpb.tolist(), "cum": cum.tolist(), "nch": nch}
    return per_core, meta, perm


def _build_program(meta):
    from concourse import bass, bacc, mybir
    import concourse.tile as tile

    f32 = mybir.dt.float32
    bf16 = mybir.dt.bfloat16
    i32 = mybir.dt.int32
    cpb, cum, nch = meta["cpb"], meta["cum"], meta["nch"]
    mxcp = max(cpb)

    nc = bacc.Bacc("TRN2", target_bir_lowering=False, debug=False,
                   dynamic_dma_scratch_size=65536)

    xe = nc.declare_dram_parameter("xe", [P, nch * DIN], bf16, isOutput=False)
    idx = nc.declare_dram_parameter("idx", [P, nch], i32, isOutput=False)
    sall = nc.declare_dram_parameter("sall", [P, nch * P], bf16, isOutput=False)
    dco = nc.declare_dram_parameter("dco", [P, NBLK], f32, isOutput=False)
    w1 = nc.declare_dram_parameter("w1", [P, DH], bf16, isOutput=False)
    w2p = nc.declare_dram_parameter("w2p", [P, 2 * DH], bf16, isOutput=False)
    w3p = nc.declare_dram_parameter("w3p", [P, 2 * DH], bf16, isOutput=False)
    bt = nc.declare_dram_parameter("bt", [P, 4], f32, isOutput=False)
    bf3 = nc.declare_dram_parameter("bf3", [P, DH], f32, isOutput=False)
    outp = nc.declare_dram_parameter("out", [SH, DH], f32, isOutput=True)

    ybin2 = nc.dram_tensor("ybin2", [SH, DH], bf16)
    ybout2 = nc.dram_tensor("ybout2", [N, DH], bf16, addr_space="Shared")
    ybin3 = nc.dram_tensor("ybin3", [SH, DH], bf16)
    ybout3 = nc.dram_tensor("ybout3", [N, DH], bf16, addr_space="Shared")

    AG = mybir.AluOpType
    ACT = mybir.ActivationFunctionType

    with tile.TileContext(nc, linearize=bool(os.environ.get("KLIN"))) as tc:
        with (
            tc.tile_pool(name="const", bufs=1) as cp_,
            tc.tile_pool(name="sb", bufs=3) as sb,
            tc.tile_pool(name="sp", bufs=4) as sp,
            tc.tile_pool(name="gp", bufs=4) as gp,
            tc.tile_pool(name="xb", bufs=2) as xbp,
            tc.tile_pool(name="pp", bufs=2, space="PSUM") as pp,
            tc.tile_pool(name="ph", bufs=6, space="PSUM") as ph,
        ):
            w1sb = cp_.tile([P, DH], dtype=bf16)
            nc.sync.dma_start(out=w1sb[:], in_=w1[:, :])
            w2sb = cp_.tile([P, 2 * DH], dtype=bf16)
            nc.sync.dma_start(out=w2sb[:], in_=w2p[:, :])
            w3sb = cp_.tile([P, 2 * DH], dtype=bf16)
            nc.sync.dma_start(out=w3sb[:], in_=w3p[:, :])
            btsb = cp_.tile([P, 4], dtype=f32)
            nc.sync.dma_start(out=btsb[:], in_=bt[:, :])
            bf3sb = cp_.tile([P, DH], dtype=f32)
            nc.sync.dma_start(out=bf3sb[:], in_=bf3[:, :])
            idxsb = cp_.tile([P, nch], dtype=i32)
            nc.sync.dma_start(out=idxsb[:], in_=idx[:, :])
            dcosb = cp_.tile([P, NBLK], dtype=f32)
            nc.sync.dma_start(out=dcosb[:], in_=dco[:, :])
            # resident transposed activations h^T: half h at cols [h*SH, ...)
            xts = cp_.tile([P, 2 * SH], dtype=bf16)

            def ld_s(b, cp):
                """Load the block's host-built scatter matrices (bf16)."""
                st = sp.tile([P, mxcp * P], dtype=bf16, tag="st")
                nc.sync.dma_start(
                    out=st[:, :cp * P],
                    in_=sall[:, cum[b] * P:(cum[b] + cp) * P])
                return st

            def phase1_win(wsb, ybin, w):
                """One window of Y = dinv * (h @ W) from xts -> ybin rows."""
                m = LASTM if w == NBLK - 1 else P
                ps = pp.tile([P, DH], dtype=f32, tag="ps")
                for h in range(2):
                    nc.tensor.matmul(
                        out=ps[:m, :],
                        lhsT=xts[:, h * SH + w * P:h * SH + w * P + m],
                        rhs=wsb[:, h * DH:(h + 1) * DH],
                        start=(h == 0), stop=(h == 1))
                ysb = sb.tile([P, DH], dtype=bf16, tag="ysb")
                nc.scalar.activation(out=ysb[:m, :], in_=ps[:m, :],
                                     func=ACT.Copy,
                                     scale=dcosb[:m, w:w + 1])
                nc.sync.dma_start(out=ybin[w * P:w * P + m, :],
                                  in_=ysb[:m, :])

            def all_gather_piece(ybin, ybout, q):
                lo, hi = AG_R[q]
                nc.gpsimd.collective_compute(
                    "AllGather", AG.bypass,
                    replica_groups=[list(range(NC))],
                    ins=[ybin[lo:hi, :].opt()],
                    outs=[ybout[NC * lo:NC * hi, :].opt()])

            # ---------------- Layer 1: streamed edge table ------------------
            for b in range(NBLK):
                cp = cpb[b]
                m = LASTM if b == NBLK - 1 else P
                xet = xbp.tile([P, mxcp * DIN], dtype=bf16, tag="xet")
                nc.sync.dma_start(
                    out=xet[:, :cp * DIN],
                    in_=xe[:, cum[b] * DIN:(cum[b] + cp) * DIN])
                st = ld_s(b, cp)
                psa = ph.tile([P, P], dtype=f32, tag="half")
                for k in range(cp):
                    nc.tensor.matmul(
                        out=psa[:, :m],
                        lhsT=xet[:, k * DIN:(k + 1) * DIN],
                        rhs=st[:, k * P:k * P + m],
                        start=(k == 0), stop=(k == cp - 1))
                agg = sb.tile([P, P], dtype=bf16, tag="agg")
                nc.scalar.activation(out=agg[:, :m], in_=psa[:, :m],
                                     func=ACT.Copy)
                psb = [ph.tile([P, P], dtype=f32, tag="half", name=f"psb{h}")
                       for h in range(2)]
                for h in range(2):
                    nc.tensor.matmul(
                        out=psb[h][:, :m],
                        lhsT=w1sb[:, h * P:(h + 1) * P],
                        rhs=agg[:, :m],
                        start=True, stop=True)
                for h in range(2):
                    nc.scalar.activation(
                        out=xts[:, h * SH + b * P:h * SH + b * P + m],
                        in_=psb[h][:, :m],
                        func=ACT.Relu, bias=btsb[:, h:h + 1])
                phase1_win(w2sb, ybin2, b)
                if b + 1 in AG_WB[1:4]:
                    all_gather_piece(ybin2, ybout2, AG_WB.index(b + 1) - 1)

            def scatter_t(table, ybin_loc, bofs, nwsb, nybin, nybout):
                """Transposed scatter + fused bias/relu epilogue -> xts,
                with the next layer's phase1 window interleaved per block."""
                for b in range(NBLK):
                    cp = cpb[b]
                    m = LASTM if b == NBLK - 1 else P
                    gt = gp.tile([P, mxcp * DH], dtype=bf16, tag="gt")
                    st = ld_s(b, cp)
                    nc.sync.dma_start(
                        out=gt[:m, (cp - 1) * DH:cp * DH],
                        in_=ybin_loc[b * P:b * P + m, :])
                    for k in range(cp - 1):
                        nc.gpsimd.indirect_dma_start(
                            out=gt[:, k * DH:(k + 1) * DH], out_offset=None,
                            in_=table[:, :],
                            in_offset=bass.IndirectOffsetOnAxis(
                                ap=idxsb[:, cum[b] + k:cum[b] + k + 1],
                                axis=0))
                    pst = [ph.tile([P, P], dtype=f32, tag="half", name=f"pst{h}")
                           for h in range(2)]
                    for k in range(cp):
                        for h in range(2):
                            nc.tensor.matmul(
                                out=pst[h][:, :m],
                                lhsT=gt[:, k * DH + h * P:k * DH + (h + 1) * P],
                                rhs=st[:, k * P:k * P + m],
                                start=(k == 0), stop=(k == cp - 1))
                    for h in range(2):
                        nc.scalar.activation(
                            out=xts[:, h * SH + b * P:h * SH + b * P + m],
                            in_=pst[h][:, :m],
                            func=ACT.Relu, bias=btsb[:, bofs + h:bofs + h + 1])
                    phase1_win(nwsb, nybin, b)
                    if b + 1 in AG_WB[1:4]:
                        all_gather_piece(nybin, nybout, AG_WB.index(b + 1) - 1)

            all_gather_piece(ybin2, ybout2, 3)
            scatter_t(ybout2, ybin2, 2, w3sb, ybin3, ybout3)
            all_gather_piece(ybin3, ybout3, 3)

            dbg = os.environ.get("KDBG")
            if dbg:
                # dump a bf16 [SH, DH] DRAM tensor to outp (cast to f32)
                src_t = {"yb2": ybin2, "yb3": ybin3}[dbg]
                for b in range(NBLK):
                    m = LASTM if b == NBLK - 1 else P
                    t = sb.tile([P, DH], dtype=bf16, tag="dbg")
                    nc.sync.dma_start(out=t[:m, :],
                                      in_=src_t[b * P:b * P + m, :])
                    t2 = sb.tile([P, DH], dtype=f32, tag="dbg2")
                    nc.vector.tensor_copy(out=t2[:m, :], in_=t[:m, :])
                    nc.sync.dma_start(out=outp[b * P:b * P + m, :],
                                      in_=t2[:m, :])

            # ---------------- Layer 3 scatter: direct [dst, feat] ----------
            for b in range(NBLK):
                cp = cpb[b]
                m = LASTM if b == NBLK - 1 else P
                gt = gp.tile([P, mxcp * DH], dtype=bf16, tag="gt")
                st = ld_s(b, cp)
                nc.sync.dma_start(
                    out=gt[:m, (cp - 1) * DH:cp * DH],
                    in_=ybin3[b * P:b * P + m, :])
                for k in range(cp - 1):
                    nc.gpsimd.indirect_dma_start(
                        out=gt[:, k * DH:(k + 1) * DH], out_offset=None,
                        in_=ybout3[:, :],
                        in_offset=bass.IndirectOffsetOnAxis(
                            ap=idxsb[:, cum[b] + k:cum[b] + k + 1], axis=0))
                ps3 = pp.tile([P, DH], dtype=f32, tag="ps")
                for k in range(cp):
                    nc.tensor.matmul(
                        out=ps3[:m, :],
                        lhsT=st[:, k * P:k * P + m],
                        rhs=gt[:, k * DH:(k + 1) * DH],
                        start=(k == 0), stop=(k == cp - 1))
                osb = sb.tile([P, DH], dtype=f32, tag="osb")
                nc.vector.tensor_tensor(out=osb[:m, :], in0=ps3[:m, :],
                                        in1=bf3sb[:m, :], op=AG.add)
                nc.sync.dma_start(out=outp[b * P:b * P + m, :],
                                  in_=osb[:m, :])

    nc.compile()
    return nc


def kernel(x, edge_index, W1, b1, W2, b2, W3, b3, _trace=False):
    from concourse.bass_utils import run_bass_kernel_spmd

    x = np.asarray(x, dtype=np.float32)
    per_core, meta, perm = _preprocess(x, edge_index)
    nc = _build_program(meta)

    w2 = np.asarray(W2, np.float32)
    w3 = np.asarray(W3, np.float32)
    w2p = np.concatenate([w2[0:P, :], w2[P:2 * P, :]], axis=1).astype(BF16)
    w3p = np.concatenate([w3[0:P, :], w3[P:2 * P, :]], axis=1).astype(BF16)
    b1v = np.asarray(b1, np.float32)
    b2v = np.asarray(b2, np.float32)
    bt = np.stack([b1v[0:P], b1v[P:2 * P], b2v[0:P], b2v[P:2 * P]],
                  axis=1).astype(np.float32)
    common = {
        "w1": np.asarray(W1, np.float32).astype(BF16),
        "w2p": w2p,
        "w3p": w3p,
        "bt": bt,
        "bf3": np.broadcast_to(np.asarray(b3, np.float32), (P, DH)).copy(),
    }
    in_maps = []
    for c in range(NC):
        m = dict(common)
        m.update(per_core[c])
        m["dco"] = per_core[c]["dco"]
        in_maps.append(m)

    res = run_bass_kernel_spmd(nc, in_maps, list(range(NC)), trace=_trace)
    shards = [res.results[c]["out"] for c in range(NC)]
    out = np.concatenate(shards, axis=0)[perm]
    if _trace:
        return out, res
    return out


# revision 32
# speedup vs baseline: 1.1518x; 1.1518x over previous
"""3-layer GCN (message passing) on 8 NeuronCores via Bass/Tile.

Strategy (vertex-cut / dst-sharding, bf16 data path):
  - Nodes are LPT-packed into (core, block) bins by in-degree so every
    128-dst block needs the same ceil(1404/128)=11 edge chunks on every
    core (SPMD program, minimal gather launches). Output rows are
    un-permuted on the host at the end.
  - Layer 1 is re-associated: relu(A_hat (X W1) + b1) = relu((A_hat X') W1
    + b1) with X' = dinv*X pre-scaled on host and expanded to edge order
    (xe), so L1 needs no indirect gathers at all - pure streaming.
  - Layers 2/3: each core computes its Y = dinv*(h @ W) shard; the table
    is AllGathered in FOUR row-pieces interleaved with the producing loop
    (gather indices are host-remapped to the piece-concatenated physical
    layout), then per-chunk indirect gathers fetch 512B bf16 rows.
    GpSimd SWDGE launch issue (~1.45us per 128-row gather) is the
    critical resource; everything else hides under it. The self-loop
    chunk of each block reads the core's OWN ybin shard rows with one
    cheap direct HWDGE DMA (core-independent address), saving one
    indirect launch per block per layer.
  - Scatter-add realized on TensorE with host-built bf16 one-hot scatter
    matrices S[e, dst] = dinv[dst] streamed per block (sall).
  - L1/L2 scatter runs transposed (psT = G^T @ S) so the ReLU epilogue
    writes h^T directly into the resident xts tile (no transposes);
    bias+relu fused into one ScalarE activation per feature half. The
    next layer's phase-1 window is interleaved after each block.
  - L3 scatter runs direct (ps = S^T @ G) to emit [dst, feat] fp32 rows.
"""

import os
import sys

sys.path.insert(0, "/opt/trn_rl_repo")

import numpy as np
import ml_dtypes

BF16 = ml_dtypes.bfloat16

N = 50000
E = 500000
NC = 8
SH = N // NC            # 6250 nodes per core
P = 128
DIN = 128
DH = 256
NBLK = (SH + P - 1) // P      # 49 dst blocks per core
LASTM = SH - (NBLK - 1) * P   # 106 dsts in the last block
AG_WB = [0, 13, 25, 37, NBLK]          # phase1 window boundaries per AG piece
AG_R = [(AG_WB[q] * P, min(AG_WB[q + 1] * P, SH)) for q in range(4)]


def _balance(deg):
    """LPT-pack nodes into (core, block) bins by in-edge weight so every
    block's edge count is ~equal across cores. Returns perm: node -> device
    row (core*SH + block*128 + slot)."""
    import heapq
    w = deg.astype(np.int64)           # regular in-edges per node
    nodes = np.argsort(-w, kind="stable")
    caps = []
    for c in range(NC):
        for b in range(NBLK):
            cap = LASTM if b == NBLK - 1 else P
            caps.append((c * SH + b * P, cap))
    heap = [(0, i) for i in range(len(caps))]
    heapq.heapify(heap)
    fill = [0] * len(caps)
    perm = np.empty(N, dtype=np.int64)
    for n in nodes:
        while True:
            wt, i = heapq.heappop(heap)
            if fill[i] < caps[i][1]:
                break
        perm[n] = caps[i][0] + fill[i]
        fill[i] += 1
        if fill[i] < caps[i][1]:
            heapq.heappush(heap, (wt + int(w[n]), i))
    return perm


def _preprocess(x, edge_index):
    """Host-side graph partitioning. Returns per-core tensors + layout meta."""
    src = np.asarray(edge_index[0], dtype=np.int64)
    dst = np.asarray(edge_index[1], dtype=np.int64)
    deg = np.bincount(dst, minlength=N).astype(np.float64)
    dinv0 = (1.0 / np.sqrt(deg + 1.0)).astype(np.float32)

    perm = _balance(deg)
    inv = np.empty(N, dtype=np.int64)
    inv[perm] = np.arange(N)           # device row -> original node

    # remap everything into device order
    src = perm[src]
    dst = perm[dst]
    x = np.asarray(x, np.float32)[inv]
    dinv = dinv0[inv]

    order = np.argsort(dst, kind="stable")
    s_s = src[order]
    d_s = dst[order]
    bounds = np.searchsorted(d_s, np.arange(NC + 1) * SH)

    # chunk counts per (core, block); cpb shared across cores (SPMD program)
    cnts = np.zeros((NC, NBLK), dtype=np.int64)
    for c in range(NC):
        lo, hi = bounds[c], bounds[c + 1]
        blk = (d_s[lo:hi] - c * SH) >> 7
        cnts[c] = np.bincount(blk, minlength=NBLK)
    cpb = np.maximum(1, (cnts.max(axis=0) + P - 1) // P) + 1  # + self chunk
    cum = np.concatenate([[0], np.cumsum(cpb)])
    nch = int(cum[-1])

    xs = (dinv[:, None] * np.asarray(x, np.float32)).astype(BF16)  # [N, 128]

    per_core = []
    for c in range(NC):
        lo, hi = bounds[c], bounds[c + 1]
        sc = s_s[lo:hi]
        dc = d_s[lo:hi] - c * SH
        blk = dc >> 7
        n_e = hi - lo
        starts = np.concatenate([[0], np.cumsum(cnts[c])])
        pos = np.arange(n_e) - starts[blk]
        col = cum[blk] + (pos >> 7)     # chunk column
        prow = pos & 127                # partition (edge slot)

        idx_all = np.zeros((P, nch), dtype=np.int32)
        # physical table row under 4-piece split-AllGather layout
        gc = sc // SH
        gr = sc % SH
        phys = np.zeros_like(sc)
        for q in range(4):
            lo, hi = AG_R[q]
            sel = (gr >= lo) & (gr < hi)
            phys[sel] = NC * lo + gc[sel] * (hi - lo) + (gr[sel] - lo)
        idx_all[prow, col] = phys.astype(np.int32)

        # host-built scatter matrices: s_all[p, col*128 + dblk] = dinv[dst]
        s_all = np.zeros((P, nch, P), dtype=np.float32)
        s_all[prow, col, dc & 127] = dinv[dc + c * SH]

        # L1 expanded edge table: xe[p, col*128 + j] = xs[src, j] (0 pads)
        xe = np.zeros((P, nch, DIN), dtype=BF16)
        xe[prow, col, :] = xs[sc]

        # self chunk (last chunk of each block): diagonal S; own rows in xe
        for b in range(NBLK):
            mm = LASTM if b == NBLK - 1 else P
            selfcol = cum[b + 1] - 1
            own = c * SH + b * P + np.arange(mm)
            s_all[np.arange(mm), selfcol, np.arange(mm)] = dinv[own]
            xe[:mm, selfcol, :] = xs[own]
        s_all = s_all.reshape(P, nch * P).astype(BF16)
        xe = xe.reshape(P, nch * DIN)

        # dinv of own shard in [p, w] window layout
        ids = c * SH + np.arange(NBLK * P)
        valid = ids < (c + 1) * SH
        dc_own = np.where(valid, dinv[np.minimum(ids, N - 1)], 0.0)
        dc_own = dc_own.reshape(NBLK, P).T.astype(np.float32).copy()

        per_core.append({
            "idx": idx_all,
            "sall": s_all,
            "xe": xe,
            "dco": dc_own,
        })

    meta = {"cpb": cpb.tolist(), "cum": cum.tolist(), "nch": nch}
    return per_core, meta, perm


def _build_program(meta):
    from concourse import bass, bacc, mybir
    import concourse.tile as tile

    f32 = mybir.dt.float32
    bf16 = mybir.dt.bfloat16
    i32 = mybir.dt.int32
    cpb, cum, nch = meta["cpb"], meta["cum"], meta["nch"]
    mxcp = max(cpb)

    nc = bacc.Bacc("TRN2", target_bir_lowering=False, debug=False,
                   dynamic_dma_scratch_size=65536)

    xe = nc.declare_dram_parameter("xe", [P, nch * DIN], bf16, isOutput=False)
    idx = nc.declare_dram_parameter("idx", [P, nch], i32, isOutput=False)
    sall = nc.declare_dram_parameter("sall", [P, nch * P], bf16, isOutput=False)
    dco = nc.declare_dram_parameter("dco", [P, NBLK], f32, isOutput=False)
    w1 = nc.declare_dram_parameter("w1", [P, DH], bf16, isOutput=False)
    w2p = nc.declare_dram_parameter("w2p", [P, 2 * DH], bf16, isOutput=False)
    w3p = nc.declare_dram_parameter("w3p", [P, 2 * DH], bf16, isOutput=False)
    bt = nc.declare_dram_parameter("bt", [P, 4], f32, isOutput=False)
    bf3 = nc.declare_dram_parameter("bf3", [P, DH], f32, isOutput=False)
    outp = nc.declare_dram_parameter("out", [SH, DH], f32, isOutput=True)

    ybin2 = nc.dram_tensor("ybin2", [SH, DH], bf16)
    ybout2 = nc.dram_tensor("ybout2", [N, DH], bf16, addr_space="Shared")
    ybin3 = nc.dram_tensor("ybin3", [SH, DH], bf16)
    ybout3 = nc.dram_tensor("ybout3", [N, DH], bf16, addr_space="Shared")

    AG = mybir.AluOpType
    ACT = mybir.ActivationFunctionType

    with tile.TileContext(nc, linearize=bool(os.environ.get("KLIN"))) as tc:
        with (
            tc.tile_pool(name="const", bufs=1) as cp_,
            tc.tile_pool(name="sb", bufs=3) as sb,
            tc.tile_pool(name="sp", bufs=4) as sp,
            tc.tile_pool(name="gp", bufs=3) as gp,
            tc.tile_pool(name="xb", bufs=2) as xbp,
            tc.tile_pool(name="pp", bufs=2, space="PSUM") as pp,
            tc.tile_pool(name="ph", bufs=6, space="PSUM") as ph,
        ):
            w1sb = cp_.tile([P, DH], dtype=bf16)
            nc.sync.dma_start(out=w1sb[:], in_=w1[:, :])
            w2sb = cp_.tile([P, 2 * DH], dtype=bf16)
            nc.sync.dma_start(out=w2sb[:], in_=w2p[:, :])
            w3sb = cp_.tile([P, 2 * DH], dtype=bf16)
            nc.sync.dma_start(out=w3sb[:], in_=w3p[:, :])
            btsb = cp_.tile([P, 4], dtype=f32)
            nc.sync.dma_start(out=btsb[:], in_=bt[:, :])
            bf3sb = cp_.tile([P, DH], dtype=f32)
            nc.sync.dma_start(out=bf3sb[:], in_=bf3[:, :])
            idxsb = cp_.tile([P, nch], dtype=i32)
            nc.sync.dma_start(out=idxsb[:], in_=idx[:, :])
            dcosb = cp_.tile([P, NBLK], dtype=f32)
            nc.sync.dma_start(out=dcosb[:], in_=dco[:, :])
            # resident transposed activations h^T: half h at cols [h*SH, ...)
            xts = cp_.tile([P, 2 * SH], dtype=bf16)

            def ld_s(b, cp):
                """Load the block's host-built scatter matrices (bf16)."""
                st = sp.tile([P, mxcp * P], dtype=bf16, tag="st")
                nc.sync.dma_start(
                    out=st[:, :cp * P],
                    in_=sall[:, cum[b] * P:(cum[b] + cp) * P])
                return st

            def phase1_win(wsb, ybin, w):
                """One window of Y = dinv * (h @ W) from xts -> ybin rows."""
                m = LASTM if w == NBLK - 1 else P
                ps = pp.tile([P, DH], dtype=f32, tag="ps")
                for h in range(2):
                    nc.tensor.matmul(
                        out=ps[:m, :],
                        lhsT=xts[:, h * SH + w * P:h * SH + w * P + m],
                        rhs=wsb[:, h * DH:(h + 1) * DH],
                        start=(h == 0), stop=(h == 1))
                ysb = sb.tile([P, DH], dtype=bf16, tag="ysb")
                nc.scalar.activation(out=ysb[:m, :], in_=ps[:m, :],
                                     func=ACT.Copy,
                                     scale=dcosb[:m, w:w + 1])
                nc.sync.dma_start(out=ybin[w * P:w * P + m, :],
                                  in_=ysb[:m, :])

            def all_gather_piece(ybin, ybout, q):
                lo, hi = AG_R[q]
                nc.gpsimd.collective_compute(
                    "AllGather", AG.bypass,
                    replica_groups=[list(range(NC))],
                    ins=[ybin[lo:hi, :].opt()],
                    outs=[ybout[NC * lo:NC * hi, :].opt()])

            # ---------------- Layer 1: streamed edge table ------------------
            for b in range(NBLK):
                cp = cpb[b]
                m = LASTM if b == NBLK - 1 else P
                xet = xbp.tile([P, mxcp * DIN], dtype=bf16, tag="xet")
                nc.sync.dma_start(
                    out=xet[:, :cp * DIN],
                    in_=xe[:, cum[b] * DIN:(cum[b] + cp) * DIN])
                st = ld_s(b, cp)
                psa = ph.tile([P, P], dtype=f32, tag="half")
                for k in range(cp):
                    nc.tensor.matmul(
                        out=psa[:, :m],
                        lhsT=xet[:, k * DIN:(k + 1) * DIN],
                        rhs=st[:, k * P:k * P + m],
                        start=(k == 0), stop=(k == cp - 1))
                agg = sb.tile([P, P], dtype=bf16, tag="agg")
                nc.scalar.activation(out=agg[:, :m], in_=psa[:, :m],
                                     func=ACT.Copy)
                psb = [ph.tile([P, P], dtype=f32, tag="half", name=f"psb{h}")
                       for h in range(2)]
                for h in range(2):
                    nc.tensor.matmul(
                        out=psb[h][:, :m],
                        lhsT=w1sb[:, h * P:(h + 1) * P],
                        rhs=agg[:, :m],
                        start=True, stop=True)
                for h in range(2):
                    nc.scalar.activation(
                        out=xts[:, h * SH + b * P:h * SH + b * P + m],
                        in_=psb[h][:, :m],
                        func=ACT.Relu, bias=btsb[:, h:h + 1])
                phase1_win(w2sb, ybin2, b)
                if b + 1 in AG_WB[1:4]:
                    all_gather_piece(ybin2, ybout2, AG_WB.index(b + 1) - 1)

            def scatter_t(table, ybin_loc, bofs, nwsb, nybin, nybout):
                """Transposed scatter + fused bias/relu epilogue -> xts,
                with the next layer's phase1 window interleaved per block."""
                for b in range(NBLK):
                    cp = cpb[b]
                    m = LASTM if b == NBLK - 1 else P
                    gt = gp.tile([P, mxcp * DH], dtype=bf16, tag="gt")
                    for k in range(cp - 1):
                        nc.gpsimd.indirect_dma_start(
                            out=gt[:, k * DH:(k + 1) * DH], out_offset=None,
                            in_=table[:, :],
                            in_offset=bass.IndirectOffsetOnAxis(
                                ap=idxsb[:, cum[b] + k:cum[b] + k + 1],
                                axis=0))
                    nc.sync.dma_start(
                        out=gt[:m, (cp - 1) * DH:cp * DH],
                        in_=ybin_loc[b * P:b * P + m, :])
                    st = ld_s(b, cp)
                    pst = [ph.tile([P, P], dtype=f32, tag="half", name=f"pst{h}")
                           for h in range(2)]
                    for k in range(cp):
                        for h in range(2):
                            nc.tensor.matmul(
                                out=pst[h][:, :m],
                                lhsT=gt[:, k * DH + h * P:k * DH + (h + 1) * P],
                                rhs=st[:, k * P:k * P + m],
                                start=(k == 0), stop=(k == cp - 1))
                    for h in range(2):
                        nc.scalar.activation(
                            out=xts[:, h * SH + b * P:h * SH + b * P + m],
                            in_=pst[h][:, :m],
                            func=ACT.Relu, bias=btsb[:, bofs + h:bofs + h + 1])
                    phase1_win(nwsb, nybin, b)
                    if b + 1 in AG_WB[1:4]:
                        all_gather_piece(nybin, nybout, AG_WB.index(b + 1) - 1)

            all_gather_piece(ybin2, ybout2, 3)
            scatter_t(ybout2, ybin2, 2, w3sb, ybin3, ybout3)
            all_gather_piece(ybin3, ybout3, 3)

            dbg = os.environ.get("KDBG")
            if dbg:
                # dump a bf16 [SH, DH] DRAM tensor to outp (cast to f32)
                src_t = {"yb2": ybin2, "yb3": ybin3}[dbg]
                for b in range(NBLK):
                    m = LASTM if b == NBLK - 1 else P
                    t = sb.tile([P, DH], dtype=bf16, tag="dbg")
                    nc.sync.dma_start(out=t[:m, :],
                                      in_=src_t[b * P:b * P + m, :])
                    t2 = sb.tile([P, DH], dtype=f32, tag="dbg2")
                    nc.vector.tensor_copy(out=t2[:m, :], in_=t[:m, :])
                    nc.sync.dma_start(out=outp[b * P:b * P + m, :],
                                      in_=t2[:m, :])

            # ---------------- Layer 3 scatter: direct [dst, feat] ----------
            for b in range(NBLK):
                cp = cpb[b]
                m = LASTM if b == NBLK - 1 else P
                gt = gp.tile([P, mxcp * DH], dtype=bf16, tag="gt")
                for k in range(cp - 1):
                    nc.gpsimd.indirect_dma_start(
                        out=gt[:, k * DH:(k + 1) * DH], out_offset=None,
                        in_=ybout3[:, :],
                        in_offset=bass.IndirectOffsetOnAxis(
                            ap=idxsb[:, cum[b] + k:cum[b] + k + 1], axis=0))
                nc.sync.dma_start(
                    out=gt[:m, (cp - 1) * DH:cp * DH],
                    in_=ybin3[b * P:b * P + m, :])
                st = ld_s(b, cp)
                ps3 = pp.tile([P, DH], dtype=f32, tag="ps")
                for k in range(cp):
                    nc.tensor.matmul(
                        out=ps3[:m, :],
                        lhsT=st[:, k * P:k * P + m],
                        rhs=gt[:, k * DH:(k + 1) * DH],
                        start=(k == 0), stop=(k == cp - 1))
                osb = sb.tile([P, DH], dtype=f32, tag="osb")
                nc.vector.tensor_tensor(out=osb[:m, :], in0=ps3[:m, :],
                                        in1=bf3sb[:m, :], op=AG.add)
                nc.sync.dma_start(out=outp[b * P:b * P + m, :],
                                  in_=osb[:m, :])

    nc.compile()
    return nc


def kernel(x, edge_index, W1, b1, W2, b2, W3, b3, _trace=False):
    from concourse.bass_utils import run_bass_kernel_spmd

    x = np.asarray(x, dtype=np.float32)
    per_core, meta, perm = _preprocess(x, edge_index)
    nc = _build_program(meta)

    w2 = np.asarray(W2, np.float32)
    w3 = np.asarray(W3, np.float32)
    w2p = np.concatenate([w2[0:P, :], w2[P:2 * P, :]], axis=1).astype(BF16)
    w3p = np.concatenate([w3[0:P, :], w3[P:2 * P, :]], axis=1).astype(BF16)
    b1v = np.asarray(b1, np.float32)
    b2v = np.asarray(b2, np.float32)
    bt = np.stack([b1v[0:P], b1v[P:2 * P], b2v[0:P], b2v[P:2 * P]],
                  axis=1).astype(np.float32)
    common = {
        "w1": np.asarray(W1, np.float32).astype(BF16),
        "w2p": w2p,
        "w3p": w3p,
        "bt": bt,
        "bf3": np.broadcast_to(np.asarray(b3, np.float32), (P, DH)).copy(),
    }
    in_maps = []
    for c in range(NC):
        m = dict(common)
        m.update(per_core[c])
        m["dco"] = per_core[c]["dco"]
        in_maps.append(m)

    res = run_bass_kernel_spmd(nc, in_maps, list(range(NC)), trace=_trace)
    shards = [res.results[c]["out"] for c in range(NC)]
    out = np.concatenate(shards, axis=0)[perm]
    if _trace:
        return out, res
    return out
